# revision 1
# baseline (speedup 1.0000x reference)
"""ALRDLinear + KIVI(2-bit key) fused kernel for one TRN2 chip (8 NeuronCores).

    y = x @ W_B^T                    [B,S,R]
    yq = kivi_qdq(y)                 per-channel 2-bit quant along token dim,
                                     groups of 128 tokens
    out = yq @ W_A^T + b_A           [B,S,O]

Sharding: tokens (B*S) are split into 8 contiguous shards of 2048 tokens.
Quantization groups (128 tokens) never straddle shard boundaries, so the
kernel needs no collectives. Weights are replicated per core.

Precision: the KIVI round() amplifies any error in y (a boundary flip costs a
full quantization step), so y is computed to ~fp32 accuracy with three
TensorEngine passes:
    A: f32r  x_hi @ w_hi      x_hi = rne12(x), w_hi = rne12(W_B^T): products
                              are exact in the PE's fp22 datapath
    B: f32r  x_lo @ w_hi      x_lo = x - x_hi (exactly representable)
    B+C: one fp8e4 DoubleRow matmul (K-pair per PE cell):
         sub-tile 0: fp8(x_lo*2^13) @ fp8(w_hi*2^6)
         sub-tile 1: fp8(x)         @ fp8(w_lo*2^19)
Pass A weights are pre-scaled by 2^19 on the host so all passes accumulate at
a common 2^19 scale in one PSUM bank; KIVI quantization is scale-invariant, so
the 2^-19 is folded into the dequantization constants. The second matmul runs
in bf16 (smooth error, no rounding amplification).
"""
import numpy as np
from contextlib import ExitStack

import concourse.bass as bass
import concourse.tile as tile
from concourse import bacc, mybir
from concourse.alu_op_type import AluOpType
from concourse.bass_utils import run_bass_kernel_spmd

F32 = mybir.dt.float32
F32R = mybir.dt.float32r
BF16 = mybir.dt.bfloat16
MAGIC = float(np.float32(2.0 ** 23))
AF = mybir.ActivationFunctionType

N_CORES = 8
B, S, D, R, O = 4, 4096, 4096, 512, 4096
TOK = B * S // N_CORES


def _build_nc(TOK=TOK, D=D, R=R, O=O, BLK=512, GRP=128,
              xt_bufs=4, yq_bufs=8, psum_y_bufs=6, psum_o_bufs=2,
              out_bufs=3, ysb_bufs=4):
    P = 128
    DC = D // P
    RB = R // P
    NB = TOK // BLK
    GPB = BLK // GRP
    OCW = 512
    OC = O // OCW
    TKC = BLK // P
    assert GRP == P

    nc = bacc.Bacc()
    xt = nc.declare_dram_parameter("xt", [D, TOK], F32, isOutput=False)
    wbt_hi = nc.declare_dram_parameter("wbt_hi", [D, R], F32, isOutput=False)
    FP8 = mybir.dt.float8e4
    wbc8 = nc.declare_dram_parameter("wbc8", [D, 2, R], FP8, isOutput=False)
    wat = nc.declare_dram_parameter("wat", [R, O], BF16, isOutput=False)
    bias = nc.declare_dram_parameter("bias", [O], F32, isOutput=False)
    out = nc.declare_dram_parameter("out", [TOK, O], F32, isOutput=True)

    with tile.TileContext(nc) as tc, ExitStack() as ctx:
        pool_w = ctx.enter_context(tc.tile_pool(name="w_persist", bufs=1))
        pool_win = ctx.enter_context(tc.tile_pool(name="w_in", bufs=3))
        pool_xt = ctx.enter_context(tc.tile_pool(name="xt", bufs=xt_bufs))
        pool_xhi = ctx.enter_context(tc.tile_pool(name="xhi", bufs=xt_bufs))
        pool_xlo = ctx.enter_context(tc.tile_pool(name="xlo", bufs=xt_bufs))
        pool_xbf = ctx.enter_context(tc.tile_pool(name="xbf", bufs=xt_bufs + 2))
        pool_t = ctx.enter_context(tc.tile_pool(name="tq", bufs=2))
        pool_ysb = ctx.enter_context(tc.tile_pool(name="ysb", bufs=ysb_bufs))
        pool_yq = ctx.enter_context(tc.tile_pool(name="yq", bufs=yq_bufs))
        pool_sm = ctx.enter_context(tc.tile_pool(name="small", bufs=6))
        pool_out = ctx.enter_context(tc.tile_pool(name="outsb", bufs=out_bufs))
        pool_py = ctx.enter_context(
            tc.tile_pool(name="psum_y", bufs=psum_y_bufs, space="PSUM"))
        pool_po = ctx.enter_context(
            tc.tile_pool(name="psum_o", bufs=psum_o_bufs, space="PSUM"))

        whi_r = [None] * DC
        wlo_sb = [None] * DC
        wat_sb = [None] * RB
        bias_sb = None

        def load_whi_chunk(c):
            w_in = pool_win.tile([P, R], F32, tag="w_in", name=f"w_in_{c}")
            nc.sync.dma_start(out=w_in, in_=wbt_hi[c * P:(c + 1) * P, :])
            w_r = pool_w.tile([P, R], F32R, tag=f"whi{c}", name=f"whi_{c}")
            nc.vector.tensor_copy(out=w_r, in_=w_in)
            whi_r[c] = w_r
            w_l = pool_w.tile([P, 2, R], FP8, tag=f"wlo{c}", name=f"wlo_{c}")
            nc.sync.dma_start(out=w_l, in_=wbc8[c * P:(c + 1) * P, :, :])
            wlo_sb[c] = w_l

        def load_mm2_weights():
            nonlocal bias_sb
            for rb in range(RB):
                w_t = pool_w.tile([P, O], BF16, tag=f"wat{rb}", name=f"wat_{rb}")
                nc.sync.dma_start(out=w_t, in_=wat[rb * P:(rb + 1) * P, :])
                wat_sb[rb] = w_t
            bias_sb = pool_w.tile([P, O], F32, tag="bias", name="bias_sb")
            bap = bias[:]
            bias_bcast = bass.AP(tensor=bap.tensor, offset=bap.offset,
                                 ap=[[0, P]] + list(bap.ap))
            nc.gpsimd.dma_start(out=bias_sb, in_=bias_bcast)

        def emit_mm1(b, mm2_groups):
            tok0 = b * BLK
            py = [pool_py.tile([P, GPB, GRP], F32, tag="py", name=f"py_{b}_{rb}")
                  for rb in range(RB)]
            for c in range(DC):
                if b == 0:
                    load_whi_chunk(c)
                if b == 1 and c == 0:
                    load_mm2_weights()
                x_in = pool_xt.tile([P, BLK], F32, tag="x_in")
                nc.sync.dma_start(out=x_in, in_=xt[c * P:(c + 1) * P, tok0:tok0 + BLK])
                x_hi = pool_xhi.tile([P, BLK], F32R, tag="x_hi")
                x_bc = pool_xbf.tile([P, 2, BLK], FP8, tag="x_bc")
                nc.vector.tensor_copy(out=x_hi, in_=x_in)
                x_lo = pool_xlo.tile([P, BLK], F32, tag="x_lo")
                nc.vector.tensor_tensor(x_lo, x_in, x_hi.bitcast(F32),
                                        AluOpType.subtract)
                nc.scalar.activation(out=x_bc[:, 0, :], in_=x_lo, func=AF.Identity,
                                     scale=8192.0)
                nc.scalar.activation(out=x_bc[:, 1, :], in_=x_in, func=AF.Identity)
                first = c == 0
                last = c == DC - 1
                for rb in range(RB):
                    w0, w1 = rb * P, (rb + 1) * P
                    nc.tensor.matmul(py[rb][:], whi_r[c][:, w0:w1], x_hi[:],
                                     start=first, stop=False)
                if mm2_groups:
                    emit_mm2_group(*mm2_groups.pop(0))
                for rb in range(RB):
                    w0, w1 = rb * P, (rb + 1) * P
                    nc.tensor.matmul(py[rb][:], wlo_sb[c][:, :, w0:w1], x_bc[:, :, :],
                                     perf_mode=mybir.MatmulPerfMode.DoubleRow,
                                     start=False, stop=last)
            return py

        def emit_quant(b, py):
            yq = []
            for rb in range(RB):
                ysb = pool_ysb.tile([P, GPB, GRP], F32, tag="ysb")
                if rb % 2 == 0:
                    nc.vector.tensor_copy(out=ysb, in_=py[rb][:])
                else:
                    nc.scalar.activation(out=ysb, in_=py[rb][:], func=AF.Identity)
                src = ysb
                mn = pool_sm.tile([P, GPB], F32, tag="mn")
                mx = pool_sm.tile([P, GPB], F32, tag="mx")
                nc.vector.tensor_reduce(mn, src[:], mybir.AxisListType.X, AluOpType.min)
                nc.vector.tensor_reduce(mx, src[:], mybir.AxisListType.X, AluOpType.max)
                diff = pool_sm.tile([P, GPB], F32, tag="diff")
                nc.vector.tensor_tensor(diff, mx, mn, AluOpType.subtract)
                scale = pool_sm.tile([P, GPB], F32, tag="scale")
                nc.vector.tensor_scalar(scale, diff, 1.0 / 3.0, 1e-8,
                                        AluOpType.mult, AluOpType.max)
                rscale = pool_sm.tile([P, GPB], F32, tag="rscale")
                nc.vector.reciprocal(out=rscale, in_=scale)
                nbias = pool_sm.tile([P, GPB], F32, tag="nbias")
                nc.vector.scalar_tensor_tensor(nbias, mn, -1.0, rscale,
                                               AluOpType.mult, AluOpType.mult)
                t = pool_t.tile([P, GPB, GRP], F32, tag="t")
                for g in range(GPB):
                    nc.scalar.activation(out=t[:, g, :], in_=src[:, g, :],
                                         func=AF.Identity,
                                         bias=nbias[:, g:g + 1],
                                         scale=rscale[:, g:g + 1])
                nc.vector.tensor_scalar(t[:], t[:], MAGIC, MAGIC,
                                        AluOpType.add, AluOpType.subtract)
                scale2 = pool_sm.tile([P, GPB], F32, tag="scale2")
                nc.vector.tensor_scalar_mul(scale2, scale, 2.0 ** -19)
                mn2 = pool_sm.tile([P, GPB], F32, tag="mn2")
                nc.vector.tensor_scalar_mul(mn2, mn, 2.0 ** -19)
                yq_t = pool_yq.tile([P, GPB, GRP], BF16, tag="yq")
                for g in range(GPB):
                    nc.scalar.activation(out=yq_t[:, g, :], in_=t[:, g, :],
                                         func=AF.Identity,
                                         bias=mn2[:, g:g + 1],
                                         scale=scale2[:, g:g + 1])
                yq.append(yq_t)
            return yq

        def emit_mm2_group(b, yq, oc, tk):
            tok0 = b * BLK
            o0, o1 = oc * OCW, (oc + 1) * OCW
            po = pool_po.tile([P, OCW], F32, tag="po", name=f"po_{b}_{oc}_{tk}")
            for rb in range(RB):
                nc.tensor.matmul(po[:], yq[rb][:, tk, :], wat_sb[rb][:, o0:o1],
                                 start=(rb == 0), stop=(rb == RB - 1))
            ob = pool_out.tile([P, OCW], F32, tag="ob", name=f"ob_{b}_{oc}_{tk}")
            nc.vector.tensor_tensor(ob, po[:], bias_sb[:, o0:o1], AluOpType.add)
            nc.sync.dma_start(out=out[tok0 + tk * P: tok0 + (tk + 1) * P, o0:o1],
                              in_=ob)

        def mm2_group_list(b, yq):
            return [(b, yq, oc, tk) for oc in range(OC) for tk in range(TKC)]

        prev = None
        for b in range(NB):
            py = emit_mm1(b, mm2_group_list(b - 1, prev) if prev is not None else [])
            prev = emit_quant(b, py)
        for g in mm2_group_list(NB - 1, prev):
            emit_mm2_group(*g)
    nc.finalize()
    return nc


def _rne12(x):
    """Round f32 to the nearest value with an 11-bit explicit mantissa (fp22)."""
    v = x.view(np.uint32).astype(np.uint64)
    half = np.uint64(1 << 11)
    add = half - np.uint64(1) + ((v >> np.uint64(12)) & np.uint64(1))
    v2 = (v + add) & np.uint64(0xFFFFF000)
    return v2.astype(np.uint32).view(np.float32)


def _make_in_maps(input, W_B, W_A, b_A):
    import ml_dtypes
    x = np.ascontiguousarray(np.asarray(input, dtype=np.float32))
    W_B = np.asarray(W_B, dtype=np.float32)
    W_A = np.asarray(W_A, dtype=np.float32)
    b_A = np.asarray(b_A, dtype=np.float32)
    Bi, Si, Di = x.shape

    toks = Bi * Si
    tok_pc = toks // N_CORES
    xf = x.reshape(toks, Di)
    wbt = np.ascontiguousarray(W_B.T).astype(np.float32)
    wh = _rne12(wbt)
    wl = (wbt - wh).astype(np.float32)
    wbt_hi = (wh * np.float32(2.0 ** 19)).astype(np.float32)
    wbc8 = np.empty(wbt.shape[:1] + (2,) + wbt.shape[1:], dtype=ml_dtypes.float8_e4m3fn)
    wbc8[:, 0, :] = np.clip(wh * np.float32(2.0 ** 6), -240, 240)
    wbc8[:, 1, :] = np.clip(wl * np.float32(2.0 ** 19), -240, 240)
    wat = np.ascontiguousarray(W_A.T).astype(ml_dtypes.bfloat16)
    in_maps = []
    for c in range(N_CORES):
        shard = np.ascontiguousarray(xf[c * tok_pc:(c + 1) * tok_pc].T)
        in_maps.append({"xt": shard, "wbt_hi": wbt_hi, "wbc8": wbc8,
                        "wat": wat, "bias": b_A})
    return in_maps, (Bi, Si, Di, W_B.shape[0], W_A.shape[0], tok_pc)


def kernel(input, W_B, W_A, b_A):
    in_maps, (Bi, Si, Di, Ri, Oi, tok_pc) = _make_in_maps(input, W_B, W_A, b_A)
    nc = _build_nc(TOK=tok_pc, D=Di, R=Ri, O=Oi)
    res = run_bass_kernel_spmd(nc, in_maps, core_ids=list(range(N_CORES)),
                               trace=False)
    out = np.concatenate([res.results[c]["out"] for c in range(N_CORES)], axis=0)
    return out.reshape(Bi, Si, Oi).astype(np.float32)



# revision 3
# speedup vs baseline: 1.3526x; 1.3526x over previous
"""ALRDLinear + KIVI(2-bit key) fused kernel for one TRN2 chip (8 NeuronCores).

    y = x @ W_B^T                    [B,S,R]
    yq = kivi_qdq(y)                 per-channel 2-bit quant along token dim,
                                     groups of 128 tokens
    out = yq @ W_A^T + b_A           [B,S,O]

Sharding: tokens (B*S) are split into 8 contiguous shards of 2048 tokens.
Quantization groups (128 tokens) never straddle shard boundaries, so the
kernel needs no collectives. Weights are replicated per core.

Precision: y is computed with a single f32r (tf32-like, 12-bit mantissa)
TensorEngine pass over host-side rne12-pre-rounded x and W_B^T. The resulting
y error (~1.5e-4 rms) shifts a small fraction of KIVI round() decisions; the
measured end-to-end relative error is ~1.3e-2, within the 2e-2 budget.
Host pre-rounding makes the engine's internal f32r rounding an identity, so
device results match the numpy model. MM2 runs in bf16 (smooth error).
"""
import numpy as np
from contextlib import ExitStack

import concourse.bass as bass
import concourse.tile as tile
from concourse import bacc, mybir
from concourse.alu_op_type import AluOpType
from concourse.bass_utils import run_bass_kernel_spmd

F32 = mybir.dt.float32
F32R = mybir.dt.float32r
BF16 = mybir.dt.bfloat16
MAGIC = float(np.float32(2.0 ** 23))
AF = mybir.ActivationFunctionType

N_CORES = 8
B, S, D, R, O = 4, 4096, 4096, 512, 4096
TOK = B * S // N_CORES


def _build_nc(TOK=TOK, D=D, R=R, O=O, BLK=512, GRP=128,
              xt_bufs=6, yq_bufs=8, psum_y_bufs=6, psum_o_bufs=2,
              out_bufs=4, ysb_bufs=4):
    P = 128
    DC = D // P
    RB = R // P
    NB = TOK // BLK
    GPB = BLK // GRP
    OCW = 512
    OC = O // OCW
    TKC = BLK // P
    assert GRP == P

    nc = bacc.Bacc()
    xt = nc.declare_dram_parameter("xt", [D, TOK], F32R, isOutput=False)
    wbt = nc.declare_dram_parameter("wbt", [D, R], F32R, isOutput=False)
    wat = nc.declare_dram_parameter("wat", [R, O], BF16, isOutput=False)
    bias = nc.declare_dram_parameter("bias", [O], F32, isOutput=False)
    out = nc.declare_dram_parameter("out", [TOK, O], F32, isOutput=True)

    with tile.TileContext(nc) as tc, ExitStack() as ctx:
        pool_w = ctx.enter_context(tc.tile_pool(name="w_persist", bufs=1))
        pool_xt = ctx.enter_context(tc.tile_pool(name="xt", bufs=xt_bufs))
        pool_t = ctx.enter_context(tc.tile_pool(name="tq", bufs=2))
        pool_ysb = ctx.enter_context(tc.tile_pool(name="ysb", bufs=ysb_bufs))
        pool_yq = ctx.enter_context(tc.tile_pool(name="yq", bufs=yq_bufs))
        pool_sm = ctx.enter_context(tc.tile_pool(name="small", bufs=6))
        pool_out = ctx.enter_context(tc.tile_pool(name="outsb", bufs=out_bufs))
        pool_py = ctx.enter_context(
            tc.tile_pool(name="psum_y", bufs=psum_y_bufs, space="PSUM"))
        pool_po = ctx.enter_context(
            tc.tile_pool(name="psum_o", bufs=psum_o_bufs, space="PSUM"))

        wr_sb = [None] * DC
        wat_sb = [None] * RB
        bias_sb = None

        def load_wr_chunk(c):
            w_t = pool_w.tile([P, R], F32R, tag=f"wr{c}", name=f"wr_{c}")
            nc.sync.dma_start(out=w_t, in_=wbt[c * P:(c + 1) * P, :])
            wr_sb[c] = w_t

        def load_mm2_weights():
            nonlocal bias_sb
            for rb in range(RB):
                w_t = pool_w.tile([P, O], BF16, tag=f"wat{rb}", name=f"wat_{rb}")
                nc.sync.dma_start(out=w_t, in_=wat[rb * P:(rb + 1) * P, :])
                wat_sb[rb] = w_t
            bias_sb = pool_w.tile([P, O], F32, tag="bias", name="bias_sb")
            bap = bias[:]
            bias_bcast = bass.AP(tensor=bap.tensor, offset=bap.offset,
                                 ap=[[0, P]] + list(bap.ap))
            nc.gpsimd.dma_start(out=bias_sb, in_=bias_bcast)

        def emit_mm1(b, mm2_groups):
            tok0 = b * BLK
            py = [pool_py.tile([P, GPB, GRP], F32, tag="py", name=f"py_{b}_{rb}")
                  for rb in range(RB)]
            for c in range(DC):
                if b == 0:
                    load_wr_chunk(c)
                if b == 1 and c == 0:
                    load_mm2_weights()
                x_in = pool_xt.tile([P, BLK], F32R, tag="x_in")
                nc.sync.dma_start(out=x_in, in_=xt[c * P:(c + 1) * P, tok0:tok0 + BLK])
                first = c == 0
                last = c == DC - 1
                for rb in range(RB):
                    w0, w1 = rb * P, (rb + 1) * P
                    nc.tensor.matmul(py[rb][:], wr_sb[c][:, w0:w1], x_in[:],
                                     start=first, stop=last)
                if mm2_groups:
                    emit_mm2_group(*mm2_groups.pop(0))
            return py

        def emit_quant(b, py):
            yq = []
            for rb in range(RB):
                ysb = pool_ysb.tile([P, GPB, GRP], F32, tag="ysb")
                if rb % 2 == 0:
                    nc.vector.tensor_copy(out=ysb, in_=py[rb][:])
                else:
                    nc.scalar.activation(out=ysb, in_=py[rb][:], func=AF.Identity)
                src = ysb
                mn = pool_sm.tile([P, GPB], F32, tag="mn")
                mx = pool_sm.tile([P, GPB], F32, tag="mx")
                nc.vector.tensor_reduce(mn, src[:], mybir.AxisListType.X, AluOpType.min)
                nc.vector.tensor_reduce(mx, src[:], mybir.AxisListType.X, AluOpType.max)
                diff = pool_sm.tile([P, GPB], F32, tag="diff")
                nc.vector.tensor_tensor(diff, mx, mn, AluOpType.subtract)
                scale = pool_sm.tile([P, GPB], F32, tag="scale")
                nc.vector.tensor_scalar(scale, diff, 1.0 / 3.0, 1e-8,
                                        AluOpType.mult, AluOpType.max)
                rscale = pool_sm.tile([P, GPB], F32, tag="rscale")
                nc.vector.reciprocal(out=rscale, in_=scale)
                nbias = pool_sm.tile([P, GPB], F32, tag="nbias")
                nc.vector.scalar_tensor_tensor(nbias, mn, -1.0, rscale,
                                               AluOpType.mult, AluOpType.mult)
                t = pool_t.tile([P, GPB, GRP], F32, tag="t")
                for g in range(GPB):
                    nc.scalar.activation(out=t[:, g, :], in_=src[:, g, :],
                                         func=AF.Identity,
                                         bias=nbias[:, g:g + 1],
                                         scale=rscale[:, g:g + 1])
                nc.vector.tensor_scalar(t[:], t[:], MAGIC, MAGIC,
                                        AluOpType.add, AluOpType.subtract)
                yq_t = pool_yq.tile([P, GPB, GRP], BF16, tag="yq")
                for g in range(GPB):
                    nc.scalar.activation(out=yq_t[:, g, :], in_=t[:, g, :],
                                         func=AF.Identity,
                                         bias=mn[:, g:g + 1],
                                         scale=scale[:, g:g + 1])
                yq.append(yq_t)
            return yq

        def emit_mm2_group(b, yq, oc, tk):
            tok0 = b * BLK
            o0, o1 = oc * OCW, (oc + 1) * OCW
            po = pool_po.tile([P, OCW], F32, tag="po", name=f"po_{b}_{oc}_{tk}")
            for rb in range(RB):
                nc.tensor.matmul(po[:], yq[rb][:, tk, :], wat_sb[rb][:, o0:o1],
                                 start=(rb == 0), stop=(rb == RB - 1))
            ob = pool_out.tile([P, OCW], F32, tag="ob", name=f"ob_{b}_{oc}_{tk}")
            nc.vector.tensor_tensor(ob, po[:], bias_sb[:, o0:o1], AluOpType.add)
            nc.sync.dma_start(out=out[tok0 + tk * P: tok0 + (tk + 1) * P, o0:o1],
                              in_=ob)

        def mm2_group_list(b, yq):
            return [(b, yq, oc, tk) for oc in range(OC) for tk in range(TKC)]

        prev = None
        for b in range(NB):
            py = emit_mm1(b, mm2_group_list(b - 1, prev) if prev is not None else [])
            prev = emit_quant(b, py)
        for g in mm2_group_list(NB - 1, prev):
            emit_mm2_group(*g)
    nc.finalize()
    return nc


def _rne12(x):
    """Round f32 to the nearest value with an 11-bit explicit mantissa (fp22)."""
    v = x.view(np.uint32).astype(np.uint64)
    half = np.uint64(1 << 11)
    add = half - np.uint64(1) + ((v >> np.uint64(12)) & np.uint64(1))
    v2 = (v + add) & np.uint64(0xFFFFF000)
    return v2.astype(np.uint32).view(np.float32)


def _make_in_maps(input, W_B, W_A, b_A):
    import ml_dtypes
    x = np.ascontiguousarray(np.asarray(input, dtype=np.float32))
    W_B = np.asarray(W_B, dtype=np.float32)
    W_A = np.asarray(W_A, dtype=np.float32)
    b_A = np.asarray(b_A, dtype=np.float32)
    Bi, Si, Di = x.shape

    toks = Bi * Si
    tok_pc = toks // N_CORES
    xf = _rne12(np.ascontiguousarray(x.reshape(toks, Di)))
    wbt = _rne12(np.ascontiguousarray(W_B.T).astype(np.float32))
    wat = np.ascontiguousarray(W_A.T).astype(ml_dtypes.bfloat16)
    in_maps = []
    for c in range(N_CORES):
        shard = np.ascontiguousarray(xf[c * tok_pc:(c + 1) * tok_pc].T)
        in_maps.append({"xt": shard, "wbt": wbt, "wat": wat, "bias": b_A})
    return in_maps, (Bi, Si, Di, W_B.shape[0], W_A.shape[0], tok_pc)


def kernel(input, W_B, W_A, b_A):
    in_maps, (Bi, Si, Di, Ri, Oi, tok_pc) = _make_in_maps(input, W_B, W_A, b_A)
    nc = _build_nc(TOK=tok_pc, D=Di, R=Ri, O=Oi)
    res = run_bass_kernel_spmd(nc, in_maps, core_ids=list(range(N_CORES)),
                               trace=False)
    out = np.concatenate([res.results[c]["out"] for c in range(N_CORES)], axis=0)
    return out.reshape(Bi, Si, Oi).astype(np.float32)


# revision 8
# speedup vs baseline: 1.3773x; 1.0182x over previous
"""ALRDLinear + KIVI(2-bit key) fused kernel for one TRN2 chip (8 NeuronCores).

    y = x @ W_B^T                    [B,S,R]
    yq = kivi_qdq(y)                 per-channel 2-bit quant along token dim,
                                     groups of 128 tokens
    out = yq @ W_A^T + b_A           [B,S,O]

Sharding: tokens (B*S) are split into 8 contiguous shards of 2048 tokens.
Quantization groups (128 tokens) never straddle shard boundaries, so the
kernel needs no collectives. Weights are replicated per core.

Precision: y is computed with a single f32r (tf32-like, 12-bit mantissa)
TensorEngine pass over host-side rne12-pre-rounded x and W_B^T. The resulting
y error (~1.5e-4 rms) shifts a small fraction of KIVI round() decisions; the
measured end-to-end relative error is ~1.3e-2, within the 2e-2 budget.
Host pre-rounding makes the engine's internal f32r rounding an identity, so
device results match the numpy model. MM2 runs in bf16 (smooth error).
"""
import numpy as np
from contextlib import ExitStack

import concourse.bass as bass
import concourse.tile as tile
from concourse import bacc, mybir
from concourse.alu_op_type import AluOpType
from concourse.bass_utils import run_bass_kernel_spmd

F32 = mybir.dt.float32
F32R = mybir.dt.float32r
BF16 = mybir.dt.bfloat16
MAGIC = float(np.float32(2.0 ** 23))
AF = mybir.ActivationFunctionType

N_CORES = 8
B, S, D, R, O = 4, 4096, 4096, 512, 4096
TOK = B * S // N_CORES


def _build_nc(TOK=TOK, D=D, R=R, O=O, BLK=512, GRP=128,
              xt_bufs=6, yq_bufs=8, psum_y_bufs=6, psum_o_bufs=2,
              out_bufs=4, ysb_bufs=4):
    P = 128
    DC = D // P
    RB = R // P
    NB = TOK // BLK
    GPB = BLK // GRP
    OCW = 512
    OC = O // OCW
    TKC = BLK // P
    assert GRP == P

    nc = bacc.Bacc()
    xt = nc.declare_dram_parameter("xt", [D, TOK], F32R, isOutput=False)
    wbt = nc.declare_dram_parameter("wbt", [D, R], F32R, isOutput=False)
    wat = nc.declare_dram_parameter("wat", [R, O], BF16, isOutput=False)
    bias = nc.declare_dram_parameter("bias", [O], F32, isOutput=False)
    out = nc.declare_dram_parameter("out", [TOK, O], BF16, isOutput=True)

    with tile.TileContext(nc) as tc, ExitStack() as ctx:
        pool_w = ctx.enter_context(tc.tile_pool(name="w_persist", bufs=1))
        pool_xt = ctx.enter_context(tc.tile_pool(name="xt", bufs=xt_bufs))
        pool_t = ctx.enter_context(tc.tile_pool(name="tq", bufs=2))
        pool_ysb = ctx.enter_context(tc.tile_pool(name="ysb", bufs=ysb_bufs))
        pool_yq = ctx.enter_context(tc.tile_pool(name="yq", bufs=yq_bufs))
        pool_sm = ctx.enter_context(tc.tile_pool(name="small", bufs=6))
        pool_out = ctx.enter_context(tc.tile_pool(name="outsb", bufs=out_bufs))
        pool_py = ctx.enter_context(
            tc.tile_pool(name="psum_y", bufs=psum_y_bufs, space="PSUM"))
        pool_po = ctx.enter_context(
            tc.tile_pool(name="psum_o", bufs=psum_o_bufs, space="PSUM"))

        wr_sb = [None] * DC
        wat_sb = [None] * RB
        bias_sb = None

        def load_wr_chunk(c):
            w_t = pool_w.tile([P, R], F32R, tag=f"wr{c}", name=f"wr_{c}")
            nc.gpsimd.dma_start(out=w_t, in_=wbt[c * P:(c + 1) * P, :])
            wr_sb[c] = w_t

        def load_mm2_weights():
            nonlocal bias_sb
            for rb in range(RB):
                w_t = pool_w.tile([P, O], BF16, tag=f"wat{rb}", name=f"wat_{rb}")
                nc.gpsimd.dma_start(out=w_t, in_=wat[rb * P:(rb + 1) * P, :])
                wat_sb[rb] = w_t
            bias_sb = pool_w.tile([P, O], F32, tag="bias", name="bias_sb")
            bap = bias[:]
            bias_bcast = bass.AP(tensor=bap.tensor, offset=bap.offset,
                                 ap=[[0, P]] + list(bap.ap))
            nc.gpsimd.dma_start(out=bias_sb, in_=bias_bcast)

        def emit_mm1(b, mm2_groups):
            tok0 = b * BLK
            py = [pool_py.tile([P, GPB, GRP], F32, tag="py", name=f"py_{b}_{rb}")
                  for rb in range(RB)]
            for c in range(DC):
                if b == 0:
                    load_wr_chunk(c)
                if b == 0 and c == DC - 4:
                    load_mm2_weights()
                x_in = pool_xt.tile([P, BLK], F32R, tag="x_in")
                nc.sync.dma_start(out=x_in, in_=xt[c * P:(c + 1) * P, tok0:tok0 + BLK])
                first = c == 0
                last = c == DC - 1
                for rb in range(RB):
                    w0, w1 = rb * P, (rb + 1) * P
                    nc.tensor.matmul(py[rb][:], wr_sb[c][:, w0:w1], x_in[:],
                                     start=first, stop=last)
                # interleave prev block's MM2 starting at chunk 8 (2 groups per
                # chunk for c=8..15, then 1 per chunk) so MM2's wat dependency
                # never head-of-line-blocks the tensor queue at block start
                if c >= 8 and mm2_groups:
                    emit_mm2_group(*mm2_groups.pop(0))
                    if c < 16 and mm2_groups:
                        emit_mm2_group(*mm2_groups.pop(0))
            return py

        def emit_quant(b, py):
            yq = []
            for rb in range(RB):
                ysb = pool_ysb.tile([P, GPB, GRP], F32, tag="ysb")
                if rb % 2 == 0:
                    nc.vector.tensor_copy(out=ysb, in_=py[rb][:])
                else:
                    nc.scalar.activation(out=ysb, in_=py[rb][:], func=AF.Identity)
                src = ysb
                mn = pool_sm.tile([P, GPB], F32, tag="mn")
                mx = pool_sm.tile([P, GPB], F32, tag="mx")
                nc.vector.tensor_reduce(mn, src[:], mybir.AxisListType.X, AluOpType.min)
                nc.vector.tensor_reduce(mx, src[:], mybir.AxisListType.X, AluOpType.max)
                diff = pool_sm.tile([P, GPB], F32, tag="diff")
                nc.vector.tensor_tensor(diff, mx, mn, AluOpType.subtract)
                scale = pool_sm.tile([P, GPB], F32, tag="scale")
                nc.vector.tensor_scalar(scale, diff, 1.0 / 3.0, 1e-8,
                                        AluOpType.mult, AluOpType.max)
                rscale = pool_sm.tile([P, GPB], F32, tag="rscale")
                nc.vector.reciprocal(out=rscale, in_=scale)
                nbias = pool_sm.tile([P, GPB], F32, tag="nbias")
                nc.vector.scalar_tensor_tensor(nbias, mn, -1.0, rscale,
                                               AluOpType.mult, AluOpType.mult)
                t = pool_t.tile([P, GPB, GRP], F32, tag="t")
                for g in range(GPB):
                    nc.scalar.activation(out=t[:, g, :], in_=src[:, g, :],
                                         func=AF.Identity,
                                         bias=nbias[:, g:g + 1],
                                         scale=rscale[:, g:g + 1])
                nc.vector.tensor_scalar(t[:], t[:], MAGIC, MAGIC,
                                        AluOpType.add, AluOpType.subtract)
                yq_t = pool_yq.tile([P, GPB, GRP], BF16, tag="yq")
                for g in range(GPB):
                    nc.scalar.activation(out=yq_t[:, g, :], in_=t[:, g, :],
                                         func=AF.Identity,
                                         bias=mn[:, g:g + 1],
                                         scale=scale[:, g:g + 1])
                yq.append(yq_t)
            return yq

        def emit_mm2_group(b, yq, oc, tk):
            tok0 = b * BLK
            o0, o1 = oc * OCW, (oc + 1) * OCW
            po = pool_po.tile([P, OCW], F32, tag="po", name=f"po_{b}_{oc}_{tk}")
            for rb in range(RB):
                nc.tensor.matmul(po[:], yq[rb][:, tk, :], wat_sb[rb][:, o0:o1],
                                 start=(rb == 0), stop=(rb == RB - 1))
            ob = pool_out.tile([P, OCW], BF16, tag="ob", name=f"ob_{b}_{oc}_{tk}")
            nc.vector.tensor_tensor(ob, po[:], bias_sb[:, o0:o1], AluOpType.add)
            nc.scalar.dma_start(out=out[tok0 + tk * P: tok0 + (tk + 1) * P, o0:o1],
                                in_=ob)

        def mm2_group_list(b, yq):
            return [(b, yq, oc, tk) for oc in range(OC) for tk in range(TKC)]

        prev = None
        for b in range(NB):
            py = emit_mm1(b, mm2_group_list(b - 1, prev) if prev is not None else [])
            prev = emit_quant(b, py)
        for g in mm2_group_list(NB - 1, prev):
            emit_mm2_group(*g)
    nc.finalize()
    return nc


def _rne12(x):
    """Round f32 to the nearest value with an 11-bit explicit mantissa (fp22)."""
    v = x.view(np.uint32).astype(np.uint64)
    half = np.uint64(1 << 11)
    add = half - np.uint64(1) + ((v >> np.uint64(12)) & np.uint64(1))
    v2 = (v + add) & np.uint64(0xFFFFF000)
    return v2.astype(np.uint32).view(np.float32)


def _make_in_maps(input, W_B, W_A, b_A):
    import ml_dtypes
    x = np.ascontiguousarray(np.asarray(input, dtype=np.float32))
    W_B = np.asarray(W_B, dtype=np.float32)
    W_A = np.asarray(W_A, dtype=np.float32)
    b_A = np.asarray(b_A, dtype=np.float32)
    Bi, Si, Di = x.shape

    toks = Bi * Si
    tok_pc = toks // N_CORES
    xf = _rne12(np.ascontiguousarray(x.reshape(toks, Di)))
    wbt = _rne12(np.ascontiguousarray(W_B.T).astype(np.float32))
    wat = np.ascontiguousarray(W_A.T).astype(ml_dtypes.bfloat16)
    in_maps = []
    for c in range(N_CORES):
        shard = np.ascontiguousarray(xf[c * tok_pc:(c + 1) * tok_pc].T)
        in_maps.append({"xt": shard, "wbt": wbt, "wat": wat, "bias": b_A})
    return in_maps, (Bi, Si, Di, W_B.shape[0], W_A.shape[0], tok_pc)


def kernel(input, W_B, W_A, b_A):
    in_maps, (Bi, Si, Di, Ri, Oi, tok_pc) = _make_in_maps(input, W_B, W_A, b_A)
    nc = _build_nc(TOK=tok_pc, D=Di, R=Ri, O=Oi)
    res = run_bass_kernel_spmd(nc, in_maps, core_ids=list(range(N_CORES)),
                               trace=False)
    out = np.concatenate([np.asarray(res.results[c]["out"]).astype(np.float32)
                          for c in range(N_CORES)], axis=0)
    return out.reshape(Bi, Si, Oi)
